# revision 1
# baseline (speedup 1.0000x reference)
"""Channel-attention kernel for Trainium2, SPMD across 8 NeuronCores.

Problem: x:[4,512,64,64] f32; q = wq@x+bq, k = wk@x+bk (Cq=64), v = wv@x+bv;
scores = q^T k -> [B,4096,4096]; attn = softmax(scores, -1);
out = v @ attn^T; y = gamma*out + x.

Sharding: 8 shards = 4 batches x 2 query-halves. Each core gets its batch's
x pre-rotated along the pixel axis so its 2048 queries sit in columns 0:2048
(softmax/AV are permutation-invariant over keys, so rotating keys/values is
harmless). This keeps the SPMD program identical on every core.

Per-core pipeline (all matmuls bf16 on the PE):
  1. Stacked QK projection (wq;wk as one [128,512] stationary -> M=128).
  2. V projection computed transposed: vT[m,c] = x^T wvT (+bv), with an
     appended ones-column so the softmax denominator falls out of the AV
     matmul for free.
  3. Scores computed transposed (scoresT[m,n] = k^T q), K=64 row-packed
     2x on the PE array; exp on the scalar engine -> bf16.
  4. AV in the [n,c] layout: outU^T[n,c] = sum_m expT[m,n] vT[m,c]; the
     ones-column yields d[n] in the same accumulation. Normalization and
     gamma fold into a per-partition activation scale.
  5. PE transpose back to [c,n] fused with the fp32 residual add.
"""

import numpy as np

import concourse.bass as bass
import concourse.bacc as bacc
import concourse.mybir as mybir
import concourse.tile as tile
from concourse import bass_utils, masks

B, C, W, H = 4, 512, 64, 64
N = W * H          # 4096 pixels
CQ = 64            # query/key channels
NH = N // 2        # 2048 queries per core
NCORES = 8
F32 = mybir.dt.float32
BF16 = mybir.dt.bfloat16
FP8E4 = mybir.dt.float8e4
FP8E5 = mybir.dt.float8e5
DR = mybir.MatmulPerfMode.DoubleRow
VPAD = 528   # fp8 vT pair stride, %16 == 0
AF = mybir.ActivationFunctionType

N_MT = N // 128    # 32 key tiles
N_G = NH // 512    # 4 query groups per core


def _emit(tc, x, wq, wk, wv, bqk, bv, gamma, y):
    nc = tc.nc
    NJ = N_MT // 2

    with (
        tc.tile_pool(name="const", bufs=1) as const,
        tc.tile_pool(name="data", bufs=1) as data,
    ):
        # ---- constants ----------------------------------------------
        id_bf = const.tile([128, 128], BF16, tag="idb")
        masks.make_identity(nc, id_bf[:])
        id_f32 = const.tile([128, 128], F32, tag="idf")
        masks.make_identity(nc, id_f32[:])
        ones_f32 = const.tile([1, 128], F32, tag="ones")
        nc.gpsimd.memset(ones_f32[:], 1.0)
        nbias = const.tile([128, 1], F32, tag="nbias")
        nc.gpsimd.memset(nbias[:], -4.0)
        onesP = const.tile([128, 32], FP8E4, tag="onesP")
        nc.gpsimd.memset(onesP[:], 1.0)

        bqk_s = const.tile([128, 1], F32, tag="bqk")
        nc.sync.dma_start(bqk_s[:], bqk)
        bv_s = const.tile([1, C], F32, tag="bvs")
        nc.sync.dma_start(bv_s[:], bv)
        g_s = const.tile([1, 1], F32, tag="gs")
        nc.sync.dma_start(g_s[:], gamma)

        bvb = const.tile([128, C], F32, tag="bvb")
        gammab = const.tile([128, 1], F32, tag="gammab")
        wqkT = [const.tile([128, 128], BF16, tag=f"wqkT{cc}", name=f"wqkT{cc}")
                for cc in range(4)]
        wvTp = [const.tile([128, 1024], FP8E4, tag=f"wvTp{pc}",
                           name=f"wvTp{pc}")
                for pc in range(2)]

        with (
            tc.tile_pool(name="wstg", bufs=2) as wstg,
            tc.tile_pool(name="pool_x", bufs=1) as pool_x,
        ):
            with tc.tile_pool(name="psA_wt", bufs=2, space="PSUM") as psA_wt:
                # gamma -> [128, 1] via K=1 matmul
                pg = psA_wt.tile([128, 1], F32, tag="wt")
                nc.tensor.matmul(pg[:], ones_f32[:], g_s[:], start=True,
                                 stop=True)
                nc.vector.tensor_copy(gammab[:], pg[:])

                # wq;wk stacked, converted to bf16, transposed on the PE
                wqk_f = wstg.tile([128, C], F32, tag="wqkf")
                nc.sync.dma_start(wqk_f[0:CQ, :], wq)
                nc.sync.dma_start(wqk_f[CQ:128, :], wk)
                wqkb = wstg.tile([128, C], BF16, tag="wqkb")
                nc.vector.tensor_copy(wqkb[:], wqk_f[:])
                for cc in range(4):
                    pt = psA_wt.tile([128, 128], BF16, tag="wt")
                    nc.tensor.transpose(pt[:], wqkb[:, cc * 128:(cc + 1) * 128],
                                        id_bf[:])
                    nc.vector.tensor_copy(wqkT[cc][:], pt[:])

                # wv -> bf16 -> wvT[cchunk][:, c_out]
                wvb = []
                for r in range(4):
                    wf = wstg.tile([128, C], F32, tag="wvf")
                    nc.sync.dma_start(wf[:], wv[r * 128:(r + 1) * 128, :])
                    wb = wstg.tile([128, C], BF16, tag="wvb", bufs=4,
                                   name=f"wvb{r}")
                    nc.vector.tensor_copy(wb[:], wf[:])
                    wvb.append(wb)
                for cc in range(4):
                    pt = psA_wt.tile([128, C], BF16, tag="wt")
                    for r in range(4):
                        nc.tensor.transpose(
                            pt[:, r * 128:(r + 1) * 128],
                            wvb[r][:, cc * 128:(cc + 1) * 128],
                            id_bf[:],
                        )
                    nc.vector.tensor_copy(
                        wvTp[cc // 2][:, (cc % 2) * 512:(cc % 2) * 512 + 512],
                        pt[:])

            with tc.tile_pool(name="psA_proj", bufs=3, space="PSUM") as psA:
                # bv -> [128, C] broadcast via K=1 matmul
                pbv = psA.tile([128, C], F32, tag="proj", name="pbv")
                nc.tensor.matmul(pbv[:], ones_f32[:], bv_s[:],
                                 start=True, stop=True)
                nc.vector.tensor_copy(bvb[:], pbv[:])

                # ---- load x via casting DMAs: bf16 (QK/scores) + fp8 (V)
                xb = [pool_x.tile([128, N], BF16, tag=f"xb{r}", name=f"xb{r}")
                      for r in range(4)]
                xp = [pool_x.tile([128, 2 * N], FP8E4, tag=f"xp{pc}",
                                  name=f"xp{pc}")
                      for pc in range(2)]
                for hh in range(2):
                    for r in range(4):
                        xsrc = x[r * 128:(r + 1) * 128, hh * NH:(hh + 1) * NH]
                        xs = wstg.tile([128, NH], F32, tag="xstg", bufs=3,
                                       name=f"xs{r}_{hh}")
                        nc.sync.dma_start(xs[:], xsrc)
                        nc.vector.tensor_copy(
                            xb[r][:, hh * NH:(hh + 1) * NH], xs[:])
                        nc.gpsimd.dma_start(
                            xp[r // 2][:, (r % 2) * N + hh * NH:
                                       (r % 2) * N + (hh + 1) * NH], xsrc)

                # ---- stacked QK projection over all pixels -----------
                qkb = pool_x.tile([128, N], BF16, tag="qkb")
                for g in range(N // 512):
                    ps = psA.tile([128, 512], F32, tag="proj")
                    for cc in range(4):
                        nc.tensor.matmul(
                            ps[:], wqkT[cc][:],
                            xb[cc][:, g * 512:(g + 1) * 512],
                            start=(cc == 0), stop=(cc == 3),
                        )
                    nc.vector.tensor_scalar_add(
                        qkb[:, g * 512:(g + 1) * 512], ps[:], bqk_s[:])

                # duplicate q (queries) and k across both partition halves
                q2 = data.tile([128, NH], BF16, tag="q2")
                k2 = data.tile([128, N], BF16, tag="k2")
                nc.sync.dma_start(q2[0:CQ, :], qkb[0:CQ, 0:NH])
                nc.sync.dma_start(q2[CQ:128, :], qkb[0:CQ, 0:NH])
                nc.sync.dma_start(k2[0:CQ, :], qkb[CQ:128, :])
                nc.sync.dma_start(k2[CQ:128, :], qkb[CQ:128, :])

            # ---- attention + V projection, shared PSUM scope ---------
            with (
                tc.tile_pool(name="psB_sc", bufs=2, space="PSUM") as psB_sc,
                tc.tile_pool(name="psB_av", bufs=3, space="PSUM") as psB_av,
                tc.tile_pool(name="psB_d", bufs=1, space="PSUM") as psB_d,
                tc.tile_pool(name="small", bufs=4) as small,
                tc.tile_pool(name="yout", bufs=3) as yout,
                tc.tile_pool(name="xres", bufs=3) as xres,
            ):
                def alloc_expP(g):
                    return [data.tile([128, 1024], FP8E5, tag=f"expP{j}",
                                      name=f"expP{j}_{g}", bufs=2)
                            for j in range(NJ)]

                def score_pair(expP_list, g, j):
                    mA, mB = 2 * j, 2 * j + 1
                    q_lo = q2[0:CQ, g * 512:(g + 1) * 512]
                    q_hi = q2[CQ:128, g * 512:(g + 1) * 512]
                    ps = psB_sc.tile([128, 1024], F32, tag="sc",
                                     name=f"ps{g}_{j}")
                    nc.tensor.matmul(
                        ps[:, 0:512], k2[0:CQ, mA * 128:(mA + 1) * 128], q_lo,
                        start=True, stop=True,
                    )
                    nc.tensor.matmul(
                        ps[:, 512:1024],
                        k2[CQ:128, mB * 128:(mB + 1) * 128], q_hi,
                        start=True, stop=True,
                    )
                    nc.scalar.activation(expP_list[j][:], ps[:], AF.Exp,
                                         bias=nbias[:])

                # group-0 scores first: ACT exps them while the PE does
                # the V projection.
                expP = alloc_expP(0)
                for j in range(NJ):
                    score_pair(expP, 0, j)

                # ---- V projection, transposed, fp8e4 pair tiles ------
                vP = [data.tile([128, 2 * VPAD], FP8E4, tag=f"vP{j}",
                                name=f"vP{j}")
                      for j in range(NJ)]
                for mt in range(N_MT):
                    ps = psB_av.tile([128, C], F32, tag="av",
                                     name=f"vps{mt}")
                    for pc in range(2):
                        lhx = xp[pc][:].rearrange("p (i n) -> p i n", i=2)[
                            :, :, mt * 128:(mt + 1) * 128]
                        wvr = wvTp[pc][:].rearrange("p (i n) -> p i n", i=2)
                        nc.tensor.matmul(
                            ps[:], lhx, wvr,
                            start=(pc == 0), stop=(pc == 1), perf_mode=DR,
                        )
                    j, half = divmod(mt, 2)
                    base = half * VPAD
                    nc.vector.tensor_add(vP[j][:, base:base + C], ps[:],
                                         bvb[:])

                # ---- groups ------------------------------------------
                for g in range(N_G):
                    nxt = alloc_expP(g + 1) if g + 1 < N_G else None
                    # softmax denominators for the whole group
                    dps = psB_d.tile([1, 512], F32, tag="d", name=f"d{g}")
                    ones_ap = onesP[:].rearrange("p (i n) -> p i n",
                                                 i=2)[:, :, 0:1]
                    for j in range(NJ):
                        nc.tensor.matmul(
                            dps[:], ones_ap,
                            expP[j][:].rearrange("p (i n) -> p i n", i=2),
                            start=(j == 0), stop=(j == NJ - 1), perf_mode=DR,
                        )
                    dsb = small.tile([1, 512], F32, tag="dsb")
                    nc.vector.tensor_copy(dsb[:], dps[:])
                    dcp = psB_d.tile([128, 4], F32, tag="d", name=f"dc{g}")
                    for t in range(4):
                        nc.tensor.matmul(
                            dcp[:, t:t + 1],
                            dsb[0:1, t * 128:(t + 1) * 128],
                            id_f32[0:1, 0:1],
                            start=True, stop=True,
                        )
                    gsc = small.tile([128, 4], F32, tag="gsc")
                    nc.vector.reciprocal(gsc[:], dcp[:])
                    nc.vector.tensor_scalar_mul(gsc[:], gsc[:], gammab[:])

                    yT = [data.tile([128, 512], BF16, tag=f"yT{t}",
                                    name=f"yT{t}_{g}")
                          for t in range(4)]
                    for t in range(4):
                        # interleave next group's scores to keep ACT fed
                        if nxt is not None:
                            for j in range(t * 4, t * 4 + 4):
                                score_pair(nxt, g + 1, j)
                        av = psB_av.tile([128, 512], F32, tag="av",
                                         name=f"av{g}_{t}")
                        for j in range(NJ):
                            lh = expP[j][:].rearrange(
                                "p (i n) -> p i n", i=2)[
                                :, :, t * 128:(t + 1) * 128]
                            vr = vP[j][:].rearrange("p (i n) -> p i n", i=2)
                            nc.tensor.matmul(
                                av[:], lh, vr[:, :, 0:512],
                                start=(j == 0), stop=(j == NJ - 1),
                                perf_mode=DR,
                            )
                        nc.vector.tensor_scalar_mul(yT[t][:], av[:],
                                                    gsc[:, t:t + 1])

                    if g < N_G - 1:
                        for cc in range(4):
                            pt = psB_sc.tile([128, 512], BF16, tag="sc",
                                             name=f"pt{g}_{cc}")
                            for t in range(4):
                                nc.tensor.transpose(
                                    pt[:, t * 128:(t + 1) * 128],
                                    yT[t][:, cc * 128:(cc + 1) * 128],
                                    id_bf[:],
                                )
                            xr = xres.tile([128, 512], F32, tag="xr")
                            nc.sync.dma_start(
                                xr[:],
                                x[cc * 128:(cc + 1) * 128,
                                  g * 512:(g + 1) * 512],
                            )
                            yo = yout.tile([128, 512], F32, tag="yo")
                            nc.vector.tensor_add(yo[:], pt[:], xr[:])
                            nc.sync.dma_start(
                                y[cc * 128:(cc + 1) * 128,
                                  g * 512:(g + 1) * 512], yo[:]
                            )
                    else:
                        # t-major: transpose strips as each yT lands so the
                        # tail after the last AV is short
                        pts = [psB_sc.tile([128, 1024], BF16, tag="sc",
                                           name=f"ptL{h}")
                               for h in range(2)]
                        for t in range(4):
                            for cc in range(4):
                                nc.tensor.transpose(
                                    pts[cc // 2][:, (cc % 2) * 512 +
                                                 t * 128:(cc % 2) * 512 +
                                                 (t + 1) * 128],
                                    yT[t][:, cc * 128:(cc + 1) * 128],
                                    id_bf[:],
                                )
                        for cc in range(4):
                            xr = xres.tile([128, 512], F32, tag="xr")
                            nc.sync.dma_start(
                                xr[:],
                                x[cc * 128:(cc + 1) * 128,
                                  g * 512:(g + 1) * 512],
                            )
                            yo = yout.tile([128, 512], F32, tag="yo")
                            nc.vector.tensor_add(
                                yo[:],
                                pts[cc // 2][:, (cc % 2) * 512:
                                             (cc % 2) * 512 + 512],
                                xr[:])
                            nc.sync.dma_start(
                                y[cc * 128:(cc + 1) * 128,
                                  g * 512:(g + 1) * 512], yo[:]
                            )
                    expP = nxt


def build_nc():
    nc = bacc.Bacc("TRN2", target_bir_lowering=False, debug=False,
                   num_devices=NCORES)
    x = nc.dram_tensor("x", [C, N], F32, kind="ExternalInput")
    wq = nc.dram_tensor("wq", [CQ, C], F32, kind="ExternalInput")
    wk = nc.dram_tensor("wk", [CQ, C], F32, kind="ExternalInput")
    wv = nc.dram_tensor("wv", [C, C], F32, kind="ExternalInput")
    bqk = nc.dram_tensor("bqk", [128, 1], F32, kind="ExternalInput")
    bv = nc.dram_tensor("bv", [1, C], F32, kind="ExternalInput")
    gamma = nc.dram_tensor("gamma", [1, 1], F32, kind="ExternalInput")
    y = nc.dram_tensor("y", [C, NH], F32, kind="ExternalOutput")
    with tile.TileContext(nc) as tc:
        _emit(tc, x.ap(), wq.ap(), wk.ap(), wv.ap(), bqk.ap(), bv.ap(),
              gamma.ap(), y.ap())
    nc.compile()
    return nc


def make_in_maps(inputs):
    xf = np.ascontiguousarray(
        np.asarray(inputs["x"], dtype=np.float32).reshape(B, C, N))
    wq = np.ascontiguousarray(np.asarray(inputs["wq"], dtype=np.float32))
    wk = np.ascontiguousarray(np.asarray(inputs["wk"], dtype=np.float32))
    wv = np.ascontiguousarray(np.asarray(inputs["wv"], dtype=np.float32))
    bqk = np.concatenate([
        np.asarray(inputs["bq"], dtype=np.float32),
        np.asarray(inputs["bk"], dtype=np.float32),
    ]).reshape(128, 1)
    bv = np.asarray(inputs["bv"], dtype=np.float32).reshape(1, C)
    gamma = np.asarray(inputs["gamma"], dtype=np.float32).reshape(1, 1)
    in_maps = []
    for i in range(NCORES):
        b, h = divmod(i, 2)
        xr = np.roll(xf[b], -h * NH, axis=1) if h else xf[b]
        in_maps.append({
            "x": np.ascontiguousarray(xr), "wq": wq, "wk": wk, "wv": wv,
            "bqk": bqk, "bv": bv, "gamma": gamma,
        })
    return in_maps


_NC = None


def _get_nc():
    global _NC
    if _NC is None:
        _NC = build_nc()
    return _NC


def kernel(**inputs):
    nc = _get_nc()
    in_maps = make_in_maps(inputs)
    res = bass_utils.run_bass_kernel_spmd(nc, in_maps, core_ids=list(range(NCORES)))
    yf = np.empty((B, C, N), dtype=np.float32)
    for i in range(NCORES):
        b, h = divmod(i, 2)
        yf[b][:, h * NH:(h + 1) * NH] = res.results[i]["y"]
    return yf.reshape(B, C, W, H)



# revision 5
# speedup vs baseline: 1.1887x; 1.1887x over previous
"""Channel-attention kernel for Trainium2, SPMD across 8 NeuronCores.

Problem: x:[4,512,64,64] f32; q = wq@x+bq, k = wk@x+bk (Cq=64), v = wv@x+bv;
scores = q^T k -> [B,4096,4096]; attn = softmax(scores, -1);
out = v @ attn^T; y = gamma*out + x.

Sharding: 8 shards = 4 batches x 2 query-halves. Each core gets its batch's
x pre-rotated along the pixel axis so its 2048 queries sit in columns 0:2048
(softmax/AV are permutation-invariant over keys, so rotating keys/values is
harmless). This keeps the SPMD program identical on every core.

Per-core pipeline (fp8 DoubleRow on the PE wherever K>=256):
  1. x is loaded ONCE, via 8 casting DMAs f32->fp8e4 split across the sync
     and gpsimd DGE rings (8 MB HBM read, the startup critical path).
     No f32/bf16 staging of x at all.
  2. QK projection in fp8 DR straight from xp (wq/wk/bq/bk are pre-scaled
     x16 on the host so the fp8e4 weights avoid the subnormal range; the
     resulting 256x score scale is removed for free by the exp activation's
     scale argument). q/k land in bf16, duplicated across both partition
     halves so score-pair matmuls run 2x concurrent on PE row-groups.
  3. V projection computed transposed (vT[m,c] = x^T wvT + bv) in fp8 DR,
     chasing the x DMA chunks half-by-half.
  4. ScoresT[m,n] = k^T q as K=64 pairs on disjoint row-groups (concurrent);
     exp on the scalar engine -> fp8e5, with scale=1/256 and bias=-4.
  5. Softmax denominators d[n] via an all-ones [128,2,128] DR stationary:
     each dps matmul broadcasts d[n] to all 128 partitions, so the
     normalization scale gamma/d is directly usable in the [c,n] layout
     (reciprocal_approx_fast + gamma on DVE). No transposes needed.
  6. AV directly in the residual layout: out[c,n] = sum_m vT[m,c] e[m,n]
     (lhsT = vP slice, rhs = expP). No output transposes.
  7. y = av * (gamma/d) + x as two DVE tensor ops; x residual tiles are
     prefetched early; output DMAs stream per 128x512 tile.

Precision: fp8 Q/K/V projections are well within the 2e-2 gate (errors
average out across the 4096-key softmax support and 512-channel
contractions); the residual path keeps x in exact fp32 end to end.
"""

import numpy as np

import concourse.bass as bass
import concourse.bacc as bacc
import concourse.mybir as mybir
import concourse.tile as tile
from concourse import bass_utils, masks

B, C, W, H = 4, 512, 64, 64
N = W * H          # 4096 pixels
CQ = 64            # query/key channels
NH = N // 2        # 2048 queries per core
NCORES = 8
F32 = mybir.dt.float32
BF16 = mybir.dt.bfloat16
FP8E4 = mybir.dt.float8e4
FP8E5 = mybir.dt.float8e5
DR = mybir.MatmulPerfMode.DoubleRow
VPAD = 528   # fp8 vT pair stride, %16 == 0
AF = mybir.ActivationFunctionType
WSCALE = 16.0          # host pre-scale on wq/wk/wv (and their biases)
ESCALE = 1.0 / (WSCALE * WSCALE)   # undone inside the exp activation

N_MT = N // 128    # 32 key tiles
N_G = NH // 512    # 4 query groups per core
NJ = N_MT // 2     # 16 fp8 pair tiles


def _emit(tc, x, wq, wk, wv, bqk, bv, gamma, y):
    nc = tc.nc

    with (
        tc.tile_pool(name="const", bufs=1) as const,
        tc.tile_pool(name="data", bufs=1) as data,
        tc.tile_pool(name="stg", bufs=2) as stg,
        tc.tile_pool(name="outp", bufs=3) as outp,
        tc.tile_pool(name="ps_sc", bufs=2, space="PSUM") as ps_sc,
        tc.tile_pool(name="ps_av", bufs=2, space="PSUM") as ps_av,
        tc.tile_pool(name="ps_d", bufs=2, space="PSUM") as ps_d,
    ):
        # ---- x fp8 cast loads first: the startup critical path ---------
        xp = [data.tile([128, 2 * N], FP8E4, tag=f"xp{pc}", name=f"xp{pc}")
              for pc in range(2)]
        for hh in range(2):
            for r in range(4):
                xsrc = x[r * 128:(r + 1) * 128, hh * NH:(hh + 1) * NH]
                dst = xp[r // 2][:, (r % 2) * N + hh * NH:
                                 (r % 2) * N + (hh + 1) * NH]
                nc.gpsimd.dma_start(dst, xsrc)

        # ---- weight/bias loads ----------------------------------------
        wqk_f = stg.tile([128, C], F32, tag="wqkf")
        nc.sync.dma_start(wqk_f[0:CQ, :], wq)
        nc.sync.dma_start(wqk_f[CQ:128, :], wk)
        wv_f = [stg.tile([128, C], F32, tag="wvf", bufs=4, name=f"wvf{r}")
                for r in range(4)]
        for r in range(4):
            nc.sync.dma_start(wv_f[r][:], wv[r * 128:(r + 1) * 128, :])
        bqk_s = const.tile([128, 1], F32, tag="bqk")
        nc.sync.dma_start(bqk_s[:], bqk)
        bv_s = const.tile([1, C], F32, tag="bvs")
        nc.sync.dma_start(bv_s[:], bv)
        g_s = const.tile([1, 1], F32, tag="gs")
        nc.sync.dma_start(g_s[:], gamma)

        # ---- x residual prefetch (sync ring) ---------------------------
        xres = [[data.tile([128, 512], F32, tag=f"xr{g}_{ct}",
                           name=f"xr{g}_{ct}") for ct in range(4)]
                for g in range(N_G)]
        for g in range(N_G):
            for ct in range(4):
                nc.sync.dma_start(
                    xres[g][ct][:],
                    x[ct * 128:(ct + 1) * 128, g * 512:(g + 1) * 512])

        # ---- constants -------------------------------------------------
        id_bf = const.tile([128, 128], BF16, tag="idb")
        masks.make_identity(nc, id_bf[:])
        ones_f32 = const.tile([1, 128], F32, tag="ones")
        nc.gpsimd.memset(ones_f32[:], 1.0)
        nbias = const.tile([128, 1], F32, tag="nbias")
        nc.gpsimd.memset(nbias[:], -4.0)
        onesDR = const.tile([128, 256], FP8E4, tag="onesDR")
        nc.gpsimd.memset(onesDR[:], 1.0)
        gammab = const.tile([128, 1], F32, tag="gammab")
        bvb = const.tile([128, C], F32, tag="bvb")

        # ---- weight prep on PE (ps_sc slots, done before scores) -------
        pg = ps_sc.tile([128, 1], F32, tag="sc", name="pg")
        nc.tensor.matmul(pg[:], ones_f32[:], g_s[:], start=True, stop=True)
        nc.vector.tensor_copy(gammab[:], pg[:])

        pbv = ps_sc.tile([128, C], F32, tag="sc", name="pbv")
        nc.tensor.matmul(pbv[:], ones_f32[:], bv_s[:], start=True, stop=True)
        nc.vector.tensor_copy(bvb[:], pbv[:])

        # wq;wk stacked -> bf16 -> transpose -> fp8 DR pairs
        wqkb = stg.tile([128, C], BF16, tag="wqkb")
        nc.vector.tensor_copy(wqkb[:], wqk_f[:])
        wqkT_dr = [const.tile([128, 256], FP8E4, tag=f"wqkT{pc}",
                              name=f"wqkT{pc}") for pc in range(2)]
        for cc in range(4):
            pt = ps_sc.tile([128, 128], BF16, tag="sc", name=f"ptq{cc}")
            nc.tensor.transpose(pt[:], wqkb[:, cc * 128:(cc + 1) * 128],
                                id_bf[:])
            nc.vector.tensor_copy(
                wqkT_dr[cc // 2][:, (cc % 2) * 128:(cc % 2) * 128 + 128],
                pt[:])

        # wv -> bf16 -> wvT fp8 pairs
        wvTp = [const.tile([128, 1024], FP8E4, tag=f"wvTp{pc}",
                           name=f"wvTp{pc}") for pc in range(2)]
        wvb = []
        for r in range(4):
            wb = stg.tile([128, C], BF16, tag="wvb", bufs=4, name=f"wvb{r}")
            nc.vector.tensor_copy(wb[:], wv_f[r][:])
            wvb.append(wb)
        for cc in range(4):
            pt = ps_sc.tile([128, C], BF16, tag="sc", name=f"ptv{cc}")
            for r in range(4):
                nc.tensor.transpose(
                    pt[:, r * 128:(r + 1) * 128],
                    wvb[r][:, cc * 128:(cc + 1) * 128],
                    id_bf[:],
                )
            nc.vector.tensor_copy(
                wvTp[cc // 2][:, (cc % 2) * 512:(cc % 2) * 512 + 512],
                pt[:])

        # ---- QK projection (fp8 DR) + V projection, chasing the DMAs --
        qkb = data.tile([128, N], BF16, tag="qkb")
        q2 = data.tile([128, NH], BF16, tag="q2")
        k2 = data.tile([128, N], BF16, tag="k2")
        vP = [data.tile([128, 2 * VPAD], FP8E4, tag=f"vP{j}", name=f"vP{j}")
              for j in range(NJ)]

        for hh in range(2):
            for gg in range(4):
                g = hh * 4 + gg
                ps = ps_av.tile([128, 512], F32, tag="av", name=f"qk{g}")
                for pc in range(2):
                    nc.tensor.matmul(
                        ps[:],
                        wqkT_dr[pc][:].rearrange("p (i n) -> p i n", i=2),
                        xp[pc][:].rearrange("p (i n) -> p i n", i=2)[
                            :, :, g * 512:(g + 1) * 512],
                        start=(pc == 0), stop=(pc == 1), perf_mode=DR,
                    )
                nc.vector.tensor_scalar_add(
                    qkb[:, g * 512:(g + 1) * 512], ps[:], bqk_s[:])
            if hh == 0:
                # queries live in pixel half 0 only
                nc.sync.dma_start(q2[0:CQ, :], qkb[0:CQ, 0:NH])
                nc.sync.dma_start(q2[CQ:128, :], qkb[0:CQ, 0:NH])
            # V projection for this half's key tiles
            for mt in range(hh * 16, hh * 16 + 16):
                ps = ps_d.tile([128, C], F32, tag="d", name=f"vps{mt}")
                for pc in range(2):
                    nc.tensor.matmul(
                        ps[:],
                        xp[pc][:].rearrange("p (i n) -> p i n", i=2)[
                            :, :, mt * 128:(mt + 1) * 128],
                        wvTp[pc][:].rearrange("p (i n) -> p i n", i=2),
                        start=(pc == 0), stop=(pc == 1), perf_mode=DR,
                    )
                j, half = divmod(mt, 2)
                nc.vector.tensor_add(vP[j][:, half * VPAD:half * VPAD + C],
                                     ps[:], bvb[:])
        nc.sync.dma_start(k2[0:CQ, :], qkb[CQ:128, :])
        nc.sync.dma_start(k2[CQ:128, :], qkb[CQ:128, :])

        # ---- attention -------------------------------------------------
        def alloc_expP(g):
            return [data.tile([128, 1024], FP8E5, tag=f"expP{j}",
                              name=f"expP{j}_{g}", bufs=2)
                    for j in range(NJ)]

        def score_pair(expP_list, g, j):
            mA, mB = 2 * j, 2 * j + 1
            q_lo = q2[0:CQ, g * 512:(g + 1) * 512]
            q_hi = q2[CQ:128, g * 512:(g + 1) * 512]
            ps = ps_sc.tile([128, 1024], F32, tag="sc", name=f"ps{g}_{j}")
            nc.tensor.matmul(
                ps[:, 0:512], k2[0:CQ, mA * 128:(mA + 1) * 128], q_lo,
                start=True, stop=True,
            )
            nc.tensor.matmul(
                ps[:, 512:1024],
                k2[CQ:128, mB * 128:(mB + 1) * 128], q_hi,
                start=True, stop=True,
            )
            nc.scalar.activation(expP_list[j][:], ps[:], AF.Exp,
                                 bias=nbias[:], scale=ESCALE)

        # group-0 scores: ACT begins its exp stream here
        expP = alloc_expP(0)
        for j in range(NJ):
            score_pair(expP, 0, j)

        for g in range(N_G):
            nxt = alloc_expP(g + 1) if g + 1 < N_G else None

            # denominators broadcast to all partitions via all-ones DR
            dsum = ps_d.tile([128, 512], F32, tag="d", name=f"dsum{g}")
            ones_ap = onesDR[:].rearrange("p (i n) -> p i n", i=2)
            for j in range(NJ):
                nc.tensor.matmul(
                    dsum[:], ones_ap,
                    expP[j][:].rearrange("p (i n) -> p i n", i=2),
                    start=(j == 0), stop=(j == NJ - 1), perf_mode=DR,
                )
            dinv = data.tile([128, 512], F32, tag="dinv", bufs=2,
                             name=f"dinv{g}")
            nc.vector.reciprocal_approx_fast(dinv[:], dsum[:])
            nc.vector.tensor_scalar_mul(dinv[:], dinv[:], gammab[:])

            for ct in range(4):
                # interleave next group's scores to keep ACT fed
                if nxt is not None:
                    for j in range(ct * 4, ct * 4 + 4):
                        score_pair(nxt, g + 1, j)
                av = ps_av.tile([128, 512], F32, tag="av",
                                name=f"av{g}_{ct}")
                for j in range(NJ):
                    nc.tensor.matmul(
                        av[:],
                        vP[j][:].rearrange("p (i n) -> p i n", i=2)[
                            :, :, ct * 128:(ct + 1) * 128],
                        expP[j][:].rearrange("p (i n) -> p i n", i=2),
                        start=(j == 0), stop=(j == NJ - 1), perf_mode=DR,
                    )
                t0 = outp.tile([128, 512], F32, tag="ysc")
                nc.vector.tensor_mul(t0[:], av[:], dinv[:])
                yo = outp.tile([128, 512], F32, tag="yo")
                nc.vector.tensor_add(yo[:], t0[:], xres[g][ct][:])
                nc.sync.dma_start(
                    y[ct * 128:(ct + 1) * 128, g * 512:(g + 1) * 512],
                    yo[:])
            expP = nxt


def build_nc():
    nc = bacc.Bacc("TRN2", target_bir_lowering=False, debug=False,
                   num_devices=NCORES)
    x = nc.dram_tensor("x", [C, N], F32, kind="ExternalInput")
    wq = nc.dram_tensor("wq", [CQ, C], F32, kind="ExternalInput")
    wk = nc.dram_tensor("wk", [CQ, C], F32, kind="ExternalInput")
    wv = nc.dram_tensor("wv", [C, C], F32, kind="ExternalInput")
    bqk = nc.dram_tensor("bqk", [128, 1], F32, kind="ExternalInput")
    bv = nc.dram_tensor("bv", [1, C], F32, kind="ExternalInput")
    gamma = nc.dram_tensor("gamma", [1, 1], F32, kind="ExternalInput")
    y = nc.dram_tensor("y", [C, NH], F32, kind="ExternalOutput")
    with tile.TileContext(nc) as tc:
        _emit(tc, x.ap(), wq.ap(), wk.ap(), wv.ap(), bqk.ap(), bv.ap(),
              gamma.ap(), y.ap())
    nc.compile()
    return nc


def make_in_maps(inputs):
    xf = np.ascontiguousarray(
        np.asarray(inputs["x"], dtype=np.float32).reshape(B, C, N))
    # WSCALE pre-scaling keeps the fp8e4 weights out of the subnormal
    # range; the score-side 256x is undone by the exp activation scale,
    # the V-side 16x by dividing gamma.
    wq = np.ascontiguousarray(
        np.asarray(inputs["wq"], dtype=np.float32) * WSCALE)
    wk = np.ascontiguousarray(
        np.asarray(inputs["wk"], dtype=np.float32) * WSCALE)
    wv = np.ascontiguousarray(
        np.asarray(inputs["wv"], dtype=np.float32) * WSCALE)
    bqk = np.concatenate([
        np.asarray(inputs["bq"], dtype=np.float32),
        np.asarray(inputs["bk"], dtype=np.float32),
    ]).reshape(128, 1) * WSCALE
    bv = np.asarray(inputs["bv"], dtype=np.float32).reshape(1, C) * WSCALE
    gamma = np.asarray(inputs["gamma"], dtype=np.float32).reshape(1, 1) / WSCALE
    in_maps = []
    for i in range(NCORES):
        b, h = divmod(i, 2)
        xr = np.roll(xf[b], -h * NH, axis=1) if h else xf[b]
        in_maps.append({
            "x": np.ascontiguousarray(xr), "wq": wq, "wk": wk, "wv": wv,
            "bqk": bqk, "bv": bv, "gamma": gamma,
        })
    return in_maps


_NC = None


def _get_nc():
    global _NC
    if _NC is None:
        _NC = build_nc()
    return _NC


def kernel(**inputs):
    nc = _get_nc()
    in_maps = make_in_maps(inputs)
    res = bass_utils.run_bass_kernel_spmd(nc, in_maps, core_ids=list(range(NCORES)))
    yf = np.empty((B, C, N), dtype=np.float32)
    for i in range(NCORES):
        b, h = divmod(i, 2)
        yf[b][:, h * NH:(h + 1) * NH] = res.results[i]["y"]
    return yf.reshape(B, C, W, H)


# revision 7
# speedup vs baseline: 1.2222x; 1.0281x over previous
"""Channel-attention kernel for Trainium2, SPMD across 8 NeuronCores.

Problem: x:[4,512,64,64] f32; q = wq@x+bq, k = wk@x+bk (Cq=64), v = wv@x+bv;
scores = q^T k -> [B,4096,4096]; attn = softmax(scores, -1);
out = v @ attn^T; y = gamma*out + x.

Sharding: 8 shards = 4 batches x 2 query-halves. Each core gets its batch's
x pre-rotated along the pixel axis so its 2048 queries sit in columns 0:2048
(softmax/AV are permutation-invariant over keys, so rotating keys/values is
harmless). This keeps the SPMD program identical on every core.

Per-core pipeline (fp8 DoubleRow on the PE wherever K>=256):
  1. x is loaded ONCE in fp8e4, split across DMA rings to hit the HBM read
     roofline: rows 256:512 go through gpsimd casting DMAs, rows 0:256 are
     staged f32 on the sync ring and cast to fp8 by the scalar engine
     (which is idle until the exp stream starts). wv rides the tensor
     engine's DGE ring so it never queues behind x.
  2. QK projection in fp8 DR straight from xp (wq/wk/bq/bk are pre-scaled
     x16 on the host so the fp8e4 weights avoid the subnormal range; the
     resulting 256x score scale is removed for free by the exp activation's
     scale argument). q/k land in bf16, duplicated across both partition
     halves so score-pair matmuls run 2x concurrent on PE row-groups.
  3. V projection computed transposed (vT[m,c] = x^T wvT) in fp8 DR,
     chasing the x chunks half-by-half. The v bias is NOT added here: it
     contributes exactly gamma*bv to y (softmax rows sum to 1), which is
     folded into the output residual add via a host-precomputed gbv tile.
  4. ScoresT[m,n] = k^T q as K=64 pairs on disjoint row-groups (concurrent);
     exp on the scalar engine -> fp8e5, with scale=1/256 and bias=-4.
  5. Softmax denominators d[n] via an all-ones [128,2,128] DR stationary:
     each dps matmul broadcasts d[n] to all 128 partitions, so the
     normalization scale gamma/d is directly usable in the [c,n] layout
     (reciprocal_approx_fast + gamma on DVE). No transposes anywhere.
  6. AV directly in the residual layout: out[c,n] = sum_m vT[m,c] e[m,n]
     (lhsT = vP slice, rhs = expP). No output transposes.
  7. y = av*(gamma/d) + gbv + x as tensor_mul + one fused
     scalar_tensor_tensor; x residual tiles are prefetched early.

Precision: fp8 Q/K/V projections are well within the 2e-2 gate (errors
average out across the 4096-key softmax support and 512-channel
contractions); the residual path keeps x in exact fp32 end to end.
"""

import numpy as np

import concourse.bass as bass
import concourse.bacc as bacc
import concourse.mybir as mybir
import concourse.tile as tile
from concourse import bass_utils, masks

B, C, W, H = 4, 512, 64, 64
N = W * H          # 4096 pixels
CQ = 64            # query/key channels
NH = N // 2        # 2048 queries per core
NCORES = 8
F32 = mybir.dt.float32
BF16 = mybir.dt.bfloat16
FP8E4 = mybir.dt.float8e4
FP8E5 = mybir.dt.float8e5
DR = mybir.MatmulPerfMode.DoubleRow
ALU = mybir.AluOpType
VPAD = 528   # fp8 vT pair stride, %16 == 0
AF = mybir.ActivationFunctionType
WSCALE = 16.0          # host pre-scale on wq/wk/wv (and q/k biases)
ESCALE = 1.0 / (WSCALE * WSCALE)   # undone inside the exp activation

N_MT = N // 128    # 32 key tiles
N_G = NH // 512    # 4 query groups per core
NJ = N_MT // 2     # 16 fp8 pair tiles


def _emit(tc, x, wq, wk, wv, bqk, gbv, gamma, y):
    nc = tc.nc

    with (
        tc.tile_pool(name="const", bufs=1) as const,
        tc.tile_pool(name="data", bufs=1) as data,
        tc.tile_pool(name="stg", bufs=2) as stg,
        tc.tile_pool(name="outp", bufs=3) as outp,
        tc.tile_pool(name="ps_sc", bufs=2, space="PSUM") as ps_sc,
        tc.tile_pool(name="ps_av", bufs=2, space="PSUM") as ps_av,
        tc.tile_pool(name="ps_d", bufs=2, space="PSUM") as ps_d,
    ):
        # ---- wv on the scalar ring: parallel with everything below ----
        wv_f = [stg.tile([128, C], F32, tag="wvf", bufs=4, name=f"wvf{r}")
                for r in range(4)]
        for r in range(4):
            nc.scalar.dma_start(wv_f[r][:], wv[r * 128:(r + 1) * 128, :])

        # ---- sync ring: small consts, x f32 staging (rows 0:256) -------
        bqk_s = const.tile([128, 1], F32, tag="bqk")
        nc.sync.dma_start(bqk_s[:], bqk)
        g_s = const.tile([1, 1], F32, tag="gs")
        nc.sync.dma_start(g_s[:], gamma)
        gbv_s = const.tile([128, 4], F32, tag="gbv")
        nc.sync.dma_start(gbv_s[:], gbv)
        wqk_f = stg.tile([128, C], F32, tag="wqkf")
        nc.sync.dma_start(wqk_f[0:CQ, :], wq)
        nc.sync.dma_start(wqk_f[CQ:128, :], wk)

        xp = [data.tile([128, 2 * N], FP8E4, tag=f"xp{pc}", name=f"xp{pc}")
              for pc in range(2)]
        xstg = []
        for hh in range(2):
            for r in range(2):
                xs = stg.tile([128, NH], F32, tag="xstg", bufs=3,
                              name=f"xs{hh}_{r}")
                nc.sync.dma_start(
                    xs[:], x[r * 128:(r + 1) * 128, hh * NH:(hh + 1) * NH])
                xstg.append((hh, r, xs))
                # rows 256:512 via gpsimd casting DMAs
                nc.gpsimd.dma_start(
                    xp[1][:, (r % 2) * N + hh * NH:(r % 2) * N + (hh + 1) * NH],
                    x[(r + 2) * 128:(r + 3) * 128, hh * NH:(hh + 1) * NH])

        # ---- x residual prefetch (sync ring, after staging) ------------
        xres = [[data.tile([128, 512], F32, tag=f"xr{g}_{ct}",
                           name=f"xr{g}_{ct}") for ct in range(4)]
                for g in range(N_G)]
        for g in range(N_G):
            for ct in range(4):
                nc.sync.dma_start(
                    xres[g][ct][:],
                    x[ct * 128:(ct + 1) * 128, g * 512:(g + 1) * 512])

        # ---- scalar engine casts the staged rows 0:256 to fp8 ----------
        for hh, r, xs in xstg:
            nc.scalar.activation(
                xp[0][:, (r % 2) * N + hh * NH:(r % 2) * N + (hh + 1) * NH],
                xs[:], AF.Copy)

        # ---- constants -------------------------------------------------
        id_bf = const.tile([128, 128], BF16, tag="idb")
        masks.make_identity(nc, id_bf[:])
        ones_f32 = const.tile([1, 128], F32, tag="ones")
        nc.gpsimd.memset(ones_f32[:], 1.0)
        nbias = const.tile([128, 1], F32, tag="nbias")
        nc.gpsimd.memset(nbias[:], -4.0)
        onesDR = const.tile([128, 256], FP8E4, tag="onesDR")
        nc.gpsimd.memset(onesDR[:], 1.0)
        gammab = const.tile([128, 1], F32, tag="gammab")

        # ---- weight prep on PE (ps_sc slots, done before scores) -------
        pg = ps_sc.tile([128, 1], F32, tag="sc", name="pg")
        nc.tensor.matmul(pg[:], ones_f32[:], g_s[:], start=True, stop=True)
        nc.vector.tensor_copy(gammab[:], pg[:])

        # wq;wk stacked -> bf16 -> transpose -> fp8 DR pairs
        wqkb = stg.tile([128, C], BF16, tag="wqkb")
        nc.vector.tensor_copy(wqkb[:], wqk_f[:])
        wqkT_dr = [const.tile([128, 256], FP8E4, tag=f"wqkT{pc}",
                              name=f"wqkT{pc}") for pc in range(2)]
        for cc in range(4):
            pt = ps_sc.tile([128, 128], BF16, tag="sc", name=f"ptq{cc}")
            nc.tensor.transpose(pt[:], wqkb[:, cc * 128:(cc + 1) * 128],
                                id_bf[:])
            nc.vector.tensor_copy(
                wqkT_dr[cc // 2][:, (cc % 2) * 128:(cc % 2) * 128 + 128],
                pt[:])

        # wv -> bf16 -> wvT fp8 pairs
        wvTp = [const.tile([128, 1024], FP8E4, tag=f"wvTp{pc}",
                           name=f"wvTp{pc}") for pc in range(2)]
        wvb = []
        for r in range(4):
            wb = stg.tile([128, C], BF16, tag="wvb", bufs=4, name=f"wvb{r}")
            nc.vector.tensor_copy(wb[:], wv_f[r][:])
            wvb.append(wb)
        for cc in range(4):
            pt = ps_sc.tile([128, C], BF16, tag="sc", name=f"ptv{cc}")
            for r in range(4):
                nc.tensor.transpose(
                    pt[:, r * 128:(r + 1) * 128],
                    wvb[r][:, cc * 128:(cc + 1) * 128],
                    id_bf[:],
                )
            nc.vector.tensor_copy(
                wvTp[cc // 2][:, (cc % 2) * 512:(cc % 2) * 512 + 512],
                pt[:])

        # ---- QK projection (fp8 DR) + V projection, chasing the DMAs --
        qkb = data.tile([128, N], BF16, tag="qkb")
        q2 = data.tile([128, NH], BF16, tag="q2")
        k2 = data.tile([128, N], BF16, tag="k2")
        vP = [data.tile([128, 2 * VPAD], FP8E4, tag=f"vP{j}", name=f"vP{j}")
              for j in range(NJ)]

        for hh in range(2):
            for gg in range(4):
                g = hh * 4 + gg
                ps = ps_av.tile([128, 512], F32, tag="av", name=f"qk{g}")
                for pc in range(2):
                    nc.tensor.matmul(
                        ps[:],
                        wqkT_dr[pc][:].rearrange("p (i n) -> p i n", i=2),
                        xp[pc][:].rearrange("p (i n) -> p i n", i=2)[
                            :, :, g * 512:(g + 1) * 512],
                        start=(pc == 0), stop=(pc == 1), perf_mode=DR,
                    )
                nc.vector.tensor_scalar_add(
                    qkb[:, g * 512:(g + 1) * 512], ps[:], bqk_s[:])
            if hh == 0:
                # queries live in pixel half 0 only
                nc.gpsimd.dma_start(q2[0:CQ, :], qkb[0:CQ, 0:NH])
                nc.gpsimd.dma_start(q2[CQ:128, :], qkb[0:CQ, 0:NH])
            # V projection for this half's key tiles
            for mt in range(hh * 16, hh * 16 + 16):
                ps = ps_d.tile([128, C], F32, tag="d", name=f"vps{mt}")
                for pc in range(2):
                    nc.tensor.matmul(
                        ps[:],
                        xp[pc][:].rearrange("p (i n) -> p i n", i=2)[
                            :, :, mt * 128:(mt + 1) * 128],
                        wvTp[pc][:].rearrange("p (i n) -> p i n", i=2),
                        start=(pc == 0), stop=(pc == 1), perf_mode=DR,
                    )
                j, half = divmod(mt, 2)
                nc.vector.tensor_copy(vP[j][:, half * VPAD:half * VPAD + C],
                                      ps[:])
        nc.gpsimd.dma_start(k2[0:CQ, :], qkb[CQ:128, :])
        nc.gpsimd.dma_start(k2[CQ:128, :], qkb[CQ:128, :])

        # ---- attention -------------------------------------------------
        def alloc_expP(g):
            return [data.tile([128, 1024], FP8E5, tag=f"expP{j}",
                              name=f"expP{j}_{g}", bufs=2)
                    for j in range(NJ)]

        def score_pair(expP_list, g, j):
            mA, mB = 2 * j, 2 * j + 1
            q_lo = q2[0:CQ, g * 512:(g + 1) * 512]
            q_hi = q2[CQ:128, g * 512:(g + 1) * 512]
            ps = ps_sc.tile([128, 1024], F32, tag="sc", name=f"ps{g}_{j}")
            nc.tensor.matmul(
                ps[:, 0:512], k2[0:CQ, mA * 128:(mA + 1) * 128], q_lo,
                start=True, stop=True,
            )
            nc.tensor.matmul(
                ps[:, 512:1024],
                k2[CQ:128, mB * 128:(mB + 1) * 128], q_hi,
                start=True, stop=True,
            )
            nc.scalar.activation(expP_list[j][:], ps[:], AF.Exp,
                                 bias=nbias[:], scale=ESCALE)

        # group-0 scores: ACT begins its exp stream here
        expP = alloc_expP(0)
        for j in range(NJ):
            score_pair(expP, 0, j)

        for g in range(N_G):
            nxt = alloc_expP(g + 1) if g + 1 < N_G else None

            # denominators broadcast to all partitions via all-ones DR
            dsum = ps_d.tile([128, 512], F32, tag="d", name=f"dsum{g}")
            ones_ap = onesDR[:].rearrange("p (i n) -> p i n", i=2)
            for j in range(NJ):
                nc.tensor.matmul(
                    dsum[:], ones_ap,
                    expP[j][:].rearrange("p (i n) -> p i n", i=2),
                    start=(j == 0), stop=(j == NJ - 1), perf_mode=DR,
                )
            dinv = data.tile([128, 512], F32, tag="dinv", bufs=2,
                             name=f"dinv{g}")
            nc.vector.reciprocal_approx_fast(dinv[:], dsum[:])
            nc.vector.tensor_scalar_mul(dinv[:], dinv[:], gammab[:])

            for ct in range(4):
                # interleave next group's scores to keep ACT fed
                if nxt is not None:
                    for j in range(ct * 4, ct * 4 + 4):
                        score_pair(nxt, g + 1, j)
                av = ps_av.tile([128, 512], F32, tag="av",
                                name=f"av{g}_{ct}")
                for j in range(NJ):
                    nc.tensor.matmul(
                        av[:],
                        vP[j][:].rearrange("p (i n) -> p i n", i=2)[
                            :, :, ct * 128:(ct + 1) * 128],
                        expP[j][:].rearrange("p (i n) -> p i n", i=2),
                        start=(j == 0), stop=(j == NJ - 1), perf_mode=DR,
                    )
                t0 = outp.tile([128, 512], F32, tag="ysc")
                nc.vector.tensor_mul(t0[:], av[:], dinv[:])
                yo = outp.tile([128, 512], F32, tag="yo")
                nc.vector.scalar_tensor_tensor(
                    yo[:], t0[:], gbv_s[:, ct:ct + 1], xres[g][ct][:],
                    ALU.add, ALU.add)
                nc.sync.dma_start(
                    y[ct * 128:(ct + 1) * 128, g * 512:(g + 1) * 512],
                    yo[:])
            expP = nxt


def build_nc():
    nc = bacc.Bacc("TRN2", target_bir_lowering=False, debug=False,
                   num_devices=NCORES)
    x = nc.dram_tensor("x", [C, N], F32, kind="ExternalInput")
    wq = nc.dram_tensor("wq", [CQ, C], F32, kind="ExternalInput")
    wk = nc.dram_tensor("wk", [CQ, C], F32, kind="ExternalInput")
    wv = nc.dram_tensor("wv", [C, C], F32, kind="ExternalInput")
    bqk = nc.dram_tensor("bqk", [128, 1], F32, kind="ExternalInput")
    gbv = nc.dram_tensor("gbv", [128, 4], F32, kind="ExternalInput")
    gamma = nc.dram_tensor("gamma", [1, 1], F32, kind="ExternalInput")
    y = nc.dram_tensor("y", [C, NH], F32, kind="ExternalOutput")
    with tile.TileContext(nc) as tc:
        _emit(tc, x.ap(), wq.ap(), wk.ap(), wv.ap(), bqk.ap(), gbv.ap(),
              gamma.ap(), y.ap())
    nc.compile()
    return nc


def make_in_maps(inputs):
    xf = np.ascontiguousarray(
        np.asarray(inputs["x"], dtype=np.float32).reshape(B, C, N))
    # WSCALE pre-scaling keeps the fp8e4 weights out of the subnormal
    # range; the score-side 256x is undone by the exp activation scale,
    # the V-side 16x by dividing gamma.
    wq = np.ascontiguousarray(
        np.asarray(inputs["wq"], dtype=np.float32) * WSCALE)
    wk = np.ascontiguousarray(
        np.asarray(inputs["wk"], dtype=np.float32) * WSCALE)
    wv = np.ascontiguousarray(
        np.asarray(inputs["wv"], dtype=np.float32) * WSCALE)
    bqk = np.concatenate([
        np.asarray(inputs["bq"], dtype=np.float32),
        np.asarray(inputs["bk"], dtype=np.float32),
    ]).reshape(128, 1) * WSCALE
    gamma_v = float(np.asarray(inputs["gamma"], dtype=np.float32).reshape(()))
    # v-bias contributes exactly gamma*bv to y (softmax rows sum to 1)
    gbv = np.ascontiguousarray(
        (gamma_v * np.asarray(inputs["bv"], dtype=np.float32))
        .reshape(4, 128).T)
    gamma = np.full((1, 1), gamma_v / WSCALE, dtype=np.float32)
    in_maps = []
    for i in range(NCORES):
        b, h = divmod(i, 2)
        xr = np.roll(xf[b], -h * NH, axis=1) if h else xf[b]
        in_maps.append({
            "x": np.ascontiguousarray(xr), "wq": wq, "wk": wk, "wv": wv,
            "bqk": bqk, "gbv": gbv, "gamma": gamma,
        })
    return in_maps


_NC = None


def _get_nc():
    global _NC
    if _NC is None:
        _NC = build_nc()
    return _NC


def kernel(**inputs):
    nc = _get_nc()
    in_maps = make_in_maps(inputs)
    res = bass_utils.run_bass_kernel_spmd(nc, in_maps, core_ids=list(range(NCORES)))
    yf = np.empty((B, C, N), dtype=np.float32)
    for i in range(NCORES):
        b, h = divmod(i, 2)
        yf[b][:, h * NH:(h + 1) * NH] = res.results[i]["y"]
    return yf.reshape(B, C, W, H)
